# revision 3
# baseline (speedup 1.0000x reference)
"""Trainium2 Bass kernel for BaseMessageModule (GNN message passing).

Strategy (v2 — SBUF-resident embedding + native Pool gather):
- Shard ATOMS across the 8 cores (3750 each). Host routes each pair to the
  core owning its receiving atom idx_i and sorts pairs by receiving atom.
- The whole embedding table lives in SBUF transposed+bf16 ([128f, 30000],
  60KB/partition) as FOUR 7500-atom tables (the Pool indirect_copy source
  is limited to ~8K indexable elements). Each pair tile (<=32 receiving
  atoms, 512 pair slots) is split into 4 bucket-chunks of 128 slots, chunk b
  holding pairs whose sending atom j falls in bucket b; per super the four
  (bucket, 6-tile) slot groups are gathered with one indirect_copy each —
  a native Pool RTL column gather, replacing the Q7 dma_gather descriptor
  generation that dominated v1 (743us of 816us).
- The gather lands [f, pair]-transposed; a per-chunk PE is_transpose matmul
  (bf16) flips it to [pair, f], 8 chunks per PSUM bank, drained to SBUF by
  one DVE copy per bank.
- Aggregation as in v1: scaled one-hot OH~[p, (k,a)] built on DVE (bf16),
  PSUM[f, (k,a)] += E.T @ OH~ per tile; W applied per-atom post-aggregation
  (linearity), bias as count*b; norms; PE transposes; output DMA.
- All matmul inputs bf16 (PSUM accumulates fp32): rel err ~1e-3 << 2e-2.

All floating-point arithmetic runs on device. Host work is integer index
manipulation (routing/sorting/padding = sharding) and array layout.
"""

import math
from contextlib import ExitStack

import numpy as np

import concourse.bass as bass
import concourse.bacc as bacc
import concourse.tile as tile
from concourse import mybir
from concourse.bass_utils import run_bass_kernel_spmd
from concourse.masks import make_identity

F = 128
ATILE = 32  # atom window per tile
KBLK = 4  # coefficient blocks: radial, u0, u1, u2
CHUNK = 128  # pairs per matmul chunk
NBUK = 4  # sending-atom buckets (= chunks per tile)
BUK = 7500  # atoms per bucket table
SUPER_T = 6  # tiles per super-iteration
N_ATOMS = 30000


def _ap(t_ap, free_dims, off=0):
    """Custom AP view over the same partitions as t_ap with given free dims."""
    return bass.AP(t_ap.tensor, t_ap.offset + off, [t_ap.ap[0]] + list(free_dims))


def build_nc(T, n_cores):
    """Build the SPMD program for one core with T pair tiles (T % 12 == 0)."""
    CH = T * NBUK  # chunks per core
    TOTP = CH * CHUNK  # padded pair slots per core
    UW = T * 3 * ATILE  # U region width, (t, c, a) order
    OUTR = T * ATILE  # output rows (tile-slot major, host depads)
    NBLK = OUTR // 128
    VW = NBLK * 128
    SUPW = SUPER_T * NBUK * CHUNK  # pair slots per super (3072)
    GW = SUPER_T * CHUNK  # slots per gather call (768)

    fp = mybir.dt.float32
    bf = mybir.dt.bfloat16

    nc = bacc.Bacc("TRN2", target_bir_lowering=False, debug=False,
                   num_devices=n_cores)

    embT = nc.dram_tensor("embT", [F, N_ATOMS], fp, kind="ExternalInput")
    jdx = nc.dram_tensor("jdx", [128, TOTP // 16], mybir.dt.uint16,
                         kind="ExternalInput")
    fT = nc.dram_tensor("fT", [128, CH], fp, kind="ExternalInput")
    r0T = nc.dram_tensor("r0T", [128, CH], fp, kind="ExternalInput")
    r1T = nc.dram_tensor("r1T", [128, CH], fp, kind="ExternalInput")
    r2T = nc.dram_tensor("r2T", [128, CH], fp, kind="ExternalInput")
    iT = nc.dram_tensor("iT", [128, CH], fp, kind="ExternalInput")
    cnt3 = nc.dram_tensor("cnt3", [1, UW], fp, kind="ExternalInput")
    wT = nc.dram_tensor("wT", [F, F], fp, kind="ExternalInput")
    brow = nc.dram_tensor("brow", [1, F], fp, kind="ExternalInput")
    out = nc.dram_tensor("out", [OUTR, 2 * F], fp, kind="ExternalOutput")

    with tile.TileContext(nc) as tc, ExitStack() as ctx:
        cpool = ctx.enter_context(tc.tile_pool(name="const", bufs=1))
        mpool = ctx.enter_context(tc.tile_pool(name="main", bufs=1))

        # --- constants ---
        iota32 = cpool.tile([128, ATILE], fp)
        nc.gpsimd.iota(iota32[:], [[1, ATILE]], base=0, channel_multiplier=0,
                       allow_small_or_imprecise_dtypes=True)
        ident_bf = cpool.tile([128, 128], bf)
        make_identity(nc, ident_bf[:])
        ident_f = cpool.tile([128, 128], fp)
        make_identity(nc, ident_f[:])
        wT_bf = cpool.tile([F, F], bf)
        brow_bf = cpool.tile([1, F], bf)
        with tc.tile_pool(name="wld", bufs=1) as wp:
            wT_sb = wp.tile([F, F], fp)
            nc.sync.dma_start(out=wT_sb[:], in_=wT[:])
            nc.scalar.copy(wT_bf[:], wT_sb[:])
            brow_sb = wp.tile([1, F], fp)
            nc.sync.dma_start(out=brow_sb[:], in_=brow[:])
            nc.scalar.copy(brow_bf[:], brow_sb[:])

        # --- resident bf16 embedding tables (4 buckets side by side) ---
        emb_bf = mpool.tile([128, NBUK * BUK], bf)
        with tc.tile_pool(name="eld", bufs=2) as lp:
            for b in range(NBUK):
                sl = lp.tile([128, BUK], fp, tag="slab")
                nc.sync.dma_start(out=sl[:], in_=embT[:, b * BUK:(b + 1) * BUK])
                nc.scalar.copy(emb_bf[:, b * BUK:(b + 1) * BUK], sl[:])

        jdx_sb = mpool.tile([128, TOTP // 16], mybir.dt.uint16)
        nc.sync.dma_start(out=jdx_sb[:], in_=jdx[:])

        # --- persistent regions ---
        iT_sb = mpool.tile([128, CH], fp)
        nc.sync.dma_start(out=iT_sb[:], in_=iT[:])
        C_all = mpool.tile([128, CH * KBLK], bf)
        U = mpool.tile([128, UW], bf)  # uw segment sums, (t, c, a)
        R2 = mpool.tile([128, VW], fp)  # radial sums, slot-major

        # --- Phase 1: batched coefficients C[p, ch, k] (scoped scratch) ---
        with tc.tile_pool(name="p1", bufs=1) as p1:
            fT_sb = p1.tile([128, CH], fp)
            nc.sync.dma_start(out=fT_sb[:], in_=fT[:])
            r0_sb = p1.tile([128, CH], fp)
            nc.sync.dma_start(out=r0_sb[:], in_=r0T[:])
            r1_sb = p1.tile([128, CH], fp)
            nc.sync.dma_start(out=r1_sb[:], in_=r1T[:])
            r2_sb = p1.tile([128, CH], fp)
            nc.sync.dma_start(out=r2_sb[:], in_=r2T[:])
            tA = p1.tile([128, CH], fp)
            tB = p1.tile([128, CH], fp)
            mult, add = mybir.AluOpType.mult, mybir.AluOpType.add
            nc.vector.tensor_tensor(out=tA[:], in0=r0_sb[:], in1=r0_sb[:], op=mult)
            nc.vector.tensor_tensor(out=tB[:], in0=r1_sb[:], in1=r1_sb[:], op=mult)
            nc.vector.tensor_tensor(out=tA[:], in0=tA[:], in1=tB[:], op=add)
            nc.vector.tensor_tensor(out=tB[:], in0=r2_sb[:], in1=r2_sb[:], op=mult)
            nc.vector.tensor_tensor(out=tA[:], in0=tA[:], in1=tB[:], op=add)
            nc.scalar.sqrt(tA[:], tA[:])  # |r|
            nc.vector.reciprocal(tB[:], tA[:])  # 1/|r|
            nc.vector.tensor_tensor(out=tB[:], in0=fT_sb[:], in1=tB[:], op=mult)
            cview = lambda k: _ap(C_all[:], [[KBLK, CH]], off=k)
            nc.vector.tensor_copy(cview(0), fT_sb[:])
            nc.vector.tensor_tensor(out=cview(1), in0=tB[:], in1=r0_sb[:], op=mult)
            nc.vector.tensor_tensor(out=cview(2), in0=tB[:], in1=r1_sb[:], op=mult)
            nc.vector.tensor_tensor(out=cview(3), in0=tB[:], in1=r2_sb[:], op=mult)

        # --- Phases 2-5 interleaved ---
        n_super = T // SUPER_T
        MMW = 512
        NP3 = math.ceil(UW / MMW)
        TPB = 128 // ATILE
        mult, add = mybir.AluOpType.mult, mybir.AluOpType.add
        with tc.tile_pool(name="gsup", bufs=2) as gpool, \
             tc.tile_pool(name="esup", bufs=2) as epool, \
             tc.tile_pool(name="osup", bufs=2) as opool, \
             tc.tile_pool(name="ptr2", bufs=2, space="PSUM") as ptpool, \
             tc.tile_pool(name="pacc", bufs=2, space="PSUM") as ppool, \
             tc.tile_pool(name="c3", bufs=2) as c3pool, \
             tc.tile_pool(name="pw", bufs=2, space="PSUM") as wpool, \
             tc.tile_pool(name="p4", bufs=2) as p4, \
             tc.tile_pool(name="ptr", bufs=2, space="PSUM") as tpool, \
             tc.tile_pool(name="ob", bufs=2) as obpool:

            def emit_p3(k):
                c0 = k * MMW
                ncol = min(MMW, UW - c0)
                c3t = c3pool.tile([1, MMW], bf, tag="c3")
                c3f = c3pool.tile([1, MMW], fp, tag="c3f")
                nc.sync.dma_start(out=c3f[:1, :ncol],
                                  in_=cnt3[:1, c0:c0 + ncol])
                nc.scalar.copy(c3t[:1, :ncol], c3f[:1, :ncol])
                pw = wpool.tile([128, MMW], fp, tag="pw")
                nc.tensor.matmul(out=pw[:, :ncol], lhsT=wT_bf[:],
                                 rhs=U[:, c0:c0 + ncol], start=True,
                                 stop=False)
                nc.tensor.matmul(out=pw[:, :ncol], lhsT=brow_bf[:1, :],
                                 rhs=c3t[:1, :ncol], start=False, stop=True)
                nc.scalar.copy(U[:, c0:c0 + ncol], pw[:, :ncol])

            def emit_blk(blk):
                t0 = blk * TPB
                s0 = p4.tile([128, 128], fp, tag="s0")
                s1 = p4.tile([128, 128], fp, tag="s1")
                vb = p4.tile([128, 128], fp, tag="vb")
                uvw = lambda c: _ap(U[:], [[3 * ATILE, TPB], [1, ATILE]],
                                    off=t0 * 3 * ATILE + c * ATILE)
                sv0 = _ap(s0[:], [[ATILE, TPB], [1, ATILE]])
                sv1 = _ap(s1[:], [[ATILE, TPB], [1, ATILE]])
                nc.vector.tensor_tensor(out=sv0, in0=uvw(0), in1=uvw(0), op=mult)
                nc.vector.tensor_tensor(out=sv1, in0=uvw(1), in1=uvw(1), op=mult)
                nc.vector.tensor_tensor(out=sv0, in0=s0[:], in1=s1[:], op=add)
                nc.vector.tensor_tensor(out=sv1, in0=uvw(2), in1=uvw(2), op=mult)
                nc.vector.tensor_tensor(out=sv0, in0=s0[:], in1=s1[:], op=add)
                nc.scalar.sqrt(vb[:], s0[:])
                ob = obpool.tile([128, 2 * F], fp, tag="ob")
                pt = tpool.tile([128, 128], fp, tag="pt")
                nc.tensor.matmul(out=pt[:], lhsT=vb[:],
                                 rhs=ident_f[:], is_transpose=True,
                                 start=True, stop=True)
                nc.scalar.copy(ob[:, 0:F], pt[:])
                pt2 = tpool.tile([128, 128], fp, tag="pt")
                nc.tensor.matmul(out=pt2[:], lhsT=R2[:, blk * 128:(blk + 1) * 128],
                                 rhs=ident_f[:], is_transpose=True,
                                 start=True, stop=True)
                nc.scalar.copy(ob[:, F:2 * F], pt2[:])
                nc.sync.dma_start(out=out[blk * 128:(blk + 1) * 128, :],
                                  in_=ob[:])

            next_p3 = 0
            next_blk = 0
            for s in range(n_super):
                sup0 = s * SUPW

                g_sup = gpool.tile([128, SUPW], bf, tag="gsup")
                for b in range(NBUK):
                    nc.gpsimd.indirect_copy(
                        g_sup[:, b * GW:(b + 1) * GW],
                        emb_bf[:, b * BUK:(b + 1) * BUK],
                        jdx_sb[:, (sup0 + b * GW) // 16:
                               (sup0 + (b + 1) * GW) // 16],
                        True,
                    )

                ch0 = s * SUPER_T * NBUK
                sc = SUPER_T * NBUK
                oh_sup = opool.tile([128, sc * ATILE], bf, tag="ohsup")
                ot_sup = opool.tile([128, sc * F], bf, tag="otsup")
                nc.vector.tensor_tensor(
                    out=_ap(oh_sup[:], [[ATILE, sc], [1, ATILE]]),
                    in0=_ap(iT_sb[:], [[1, sc], [0, ATILE]], off=ch0),
                    in1=_ap(iota32[:], [[0, sc], [1, ATILE]]),
                    op=mybir.AluOpType.is_equal,
                )
                nc.vector.tensor_tensor(
                    out=_ap(ot_sup[:], [[F, sc], [ATILE, KBLK], [1, ATILE]]),
                    in0=_ap(oh_sup[:], [[ATILE, sc], [0, KBLK], [1, ATILE]]),
                    in1=_ap(C_all[:], [[KBLK, sc], [1, KBLK], [0, ATILE]],
                            off=ch0 * KBLK),
                    op=mybir.AluOpType.mult,
                )

                # transpose gathered [f, p] chunks to [p, f], 8 per PSUM bank
                e_sup = epool.tile([128, sc * F], bf, tag="esup")
                for q in range(sc // 8):
                    pt8 = ptpool.tile([128, 8 * 128], bf, tag="pt8")
                    for c in range(8):
                        ch = q * 8 + c
                        nc.tensor.matmul(
                            out=pt8[:, c * 128:(c + 1) * 128],
                            lhsT=g_sup[:, ch * 128:(ch + 1) * 128],
                            rhs=ident_bf[:], is_transpose=True,
                            start=True, stop=True,
                        )
                    nc.vector.tensor_copy(
                        e_sup[:, q * 1024:(q + 1) * 1024], pt8[:])

                for ti in range(SUPER_T):
                    t = s * SUPER_T + ti
                    acc = ppool.tile([128, F], fp, tag="acc")
                    for b in range(NBUK):
                        g = b * SUPER_T + ti
                        nc.tensor.matmul(
                            out=acc[:],
                            lhsT=_ap(e_sup[:], [[1, F]], off=g * F),
                            rhs=_ap(ot_sup[:], [[1, F]], off=g * F),
                            start=(b == 0),
                            stop=(b == NBUK - 1),
                        )
                    nc.scalar.copy(R2[:, t * ATILE:(t + 1) * ATILE],
                                   acc[:, 0:ATILE])
                    nc.scalar.copy(U[:, t * 3 * ATILE:(t + 1) * 3 * ATILE],
                                   acc[:, ATILE:F])

                # trailing work whose inputs are now flushed
                flushed_cols = (s + 1) * SUPER_T * 3 * ATILE
                while next_p3 < NP3 and (
                        (next_p3 + 1) * MMW <= flushed_cols
                        or s + 1 == n_super):
                    emit_p3(next_p3)
                    next_p3 += 1
                while next_blk < NBLK and (next_blk + 1) * TPB * 3 * ATILE \
                        <= next_p3 * MMW:
                    emit_blk(next_blk)
                    next_blk += 1
            while next_blk < NBLK:
                emit_blk(next_blk)
                next_blk += 1

    nc.compile()
    return nc


def host_prep(inputs, n_cores=8):
    """Route pairs to atom-owning cores; 32-atom tiles with 4 bucket-chunks."""
    emb = np.ascontiguousarray(np.asarray(inputs["atomic_embedding"],
                                          dtype=np.float32))
    f = np.asarray(inputs["f_ij_cutoff"], dtype=np.float32).ravel()
    r = np.asarray(inputs["r_ij"], dtype=np.float32)
    W = np.asarray(inputs["W"], dtype=np.float32)
    b = np.asarray(inputs["b"], dtype=np.float32)
    pl = np.asarray(inputs["pairlist"]).astype(np.int64)
    idx_i, idx_j = pl[0], pl[1]

    N = emb.shape[0]
    P = idx_i.shape[0]
    APC = N // n_cores
    SLOTS = NBUK * CHUNK  # pair slots per tile (512)
    buk_of_pair = idx_j // BUK

    # per-atom-per-bucket counts [N, NBUK]
    cnt_ab = np.zeros((N, NBUK), dtype=np.int64)
    np.add.at(cnt_ab, (idx_i, buk_of_pair), 1)
    cnt_atom = cnt_ab.sum(axis=1)

    # greedy variable-base tiling per core: close a tile when any bucket
    # chunk (128 slots) would overflow or the atom window exceeds ATILE
    tiles = []
    for c in range(n_cores):
        ca = cnt_ab[c * APC:(c + 1) * APC]
        tl = []
        cur = 0
        cur_b = np.zeros(NBUK, dtype=np.int64)
        for a in range(APC):
            cb = ca[a]
            if (cur_b + cb > CHUNK).any() or a - cur >= ATILE:
                tl.append((cur, a))
                cur = a
                cur_b[:] = 0
            cur_b += cb
        tl.append((cur, APC))
        tiles.append(tl)
    T = max(len(tl) for tl in tiles)
    T = ((T + 11) // 12) * 12  # multiple of SUPER_T(6) and 4

    tile_of_atom = np.zeros(N, dtype=np.int64)
    base_of_atom = np.zeros(N, dtype=np.int64)
    for c in range(n_cores):
        for t, (a0, a1) in enumerate(tiles[c]):
            tile_of_atom[c * APC + a0:c * APC + a1] = t
            base_of_atom[c * APC + a0:c * APC + a1] = a0

    # slot layout: super s => [bucket b => [tile ti => 128 slots]]
    # global chunk index g = s*24 + b*6 + ti ; slot = g*128 + pos
    order = np.argsort(idx_i, kind="stable")
    so_i = idx_i[order]
    so_b = buk_of_pair[order]
    core_of = so_i // APC
    t_of = tile_of_atom[so_i]
    s_of = t_of // SUPER_T
    ti_of = t_of % SUPER_T
    chunk_of = s_of * (SUPER_T * NBUK) + so_b * SUPER_T + ti_of
    key = core_of * (T * NBUK) + chunk_of
    cnt = np.bincount(key, minlength=n_cores * T * NBUK)
    assert cnt.max() <= CHUNK, cnt.max()
    starts = np.zeros(n_cores * T * NBUK + 1, dtype=np.int64)
    np.cumsum(cnt, out=starts[1:])
    # order within (core, chunk): stable sort by key keeps i-sorted order
    order2 = np.argsort(key, kind="stable")
    pos = np.arange(P, dtype=np.int64)
    pos_in_chunk = pos - starts[key[order2]]
    slot = np.empty(P, dtype=np.int64)
    slot[order2] = key[order2] * CHUNK + pos_in_chunk
    TOT = n_cores * T * NBUK * CHUNK

    jj = np.zeros(TOT, dtype=np.uint16)
    ff = np.zeros(TOT, dtype=np.float32)
    rr = np.zeros((TOT, 3), dtype=np.float32)
    rr[:, 0] = 1.0
    ii = np.zeros(TOT, dtype=np.float32)
    jj[slot] = (idx_j[order] - so_b * BUK).astype(np.uint16)
    ff[slot] = f[order]
    rr[slot] = r[order]
    ii[slot] = (so_i - core_of * APC - base_of_atom[so_i]).astype(np.float32)

    TOTC = T * NBUK * CHUNK  # padded pair slots per core
    CH = T * NBUK
    in_maps = []
    out_sel = []
    embT = np.ascontiguousarray(emb.T)
    wTc = np.ascontiguousarray(W.T)
    browc = np.ascontiguousarray(b.reshape(1, F))
    for c in range(n_cores):
        sl = slice(c * TOTC, (c + 1) * TOTC)
        jj_c = jj[sl]
        a16 = np.ascontiguousarray(jj_c.reshape(TOTC // 16, 16).T)
        jdx = np.ascontiguousarray(np.tile(a16, (8, 1)))
        tr = lambda x: np.ascontiguousarray(x.reshape(CH, CHUNK).T)
        cnt3 = np.zeros((T, 3, ATILE), dtype=np.float32)
        rows_slot = []
        rows_atom = []
        for t, (a0, a1) in enumerate(tiles[c]):
            span = a1 - a0
            cnt3[t, :, :span] = cnt_atom[c * APC + a0:c * APC + a1][None, :]
            rows_slot.append(np.arange(t * ATILE, t * ATILE + span))
            rows_atom.append(np.arange(c * APC + a0, c * APC + a1))
        out_sel.append((np.concatenate(rows_slot), np.concatenate(rows_atom)))
        in_maps.append({
            "embT": embT,
            "jdx": jdx,
            "fT": tr(ff[sl]),
            "r0T": tr(rr[sl][:, 0]),
            "r1T": tr(rr[sl][:, 1]),
            "r2T": tr(rr[sl][:, 2]),
            "iT": tr(ii[sl]),
            "cnt3": np.ascontiguousarray(cnt3.reshape(1, -1)),
            "wT": wTc,
            "brow": browc,
        })
    return in_maps, dict(N=N, APC=APC, T=T, P=P, out_sel=out_sel)


_NC_CACHE = {}


def kernel(**inputs) -> np.ndarray:
    n_cores = 8
    in_maps, meta = host_prep(inputs, n_cores)
    N = meta["N"]
    ckey = (N, meta["T"], n_cores)
    nc = _NC_CACHE.get(ckey)
    if nc is None:
        nc = build_nc(meta["T"], n_cores)
        _NC_CACHE[ckey] = nc
    res = run_bass_kernel_spmd(nc, in_maps, core_ids=list(range(n_cores)))
    out = np.empty((N, 2 * F), dtype=np.float32)
    for c in range(n_cores):
        rows_slot, rows_atom = meta["out_sel"][c]
        out[rows_atom] = res.results[c]["out"][rows_slot]
    return out


# revision 6
# speedup vs baseline: 2.7068x; 2.7068x over previous
"""Trainium2 Bass kernel for BaseMessageModule (GNN message passing).

Strategy (v2 — SBUF-resident embedding + native Pool gather):
- Shard ATOMS across the 8 cores (3750 each). Host routes each pair to the
  core owning its receiving atom idx_i and sorts pairs by receiving atom.
- The whole embedding table lives in SBUF transposed+bf16 ([128f, 30000],
  60KB/partition) as FOUR 7500-atom tables (the Pool indirect_copy source
  is limited to ~8K indexable elements). Each pair tile (<=32 receiving
  atoms, 512 pair slots) is split into 4 bucket-chunks of 128 slots, chunk b
  holding pairs whose sending atom j falls in bucket b; per super the four
  (bucket, 6-tile) slot groups are gathered with one indirect_copy each —
  a native Pool RTL column gather, replacing the Q7 dma_gather descriptor
  generation that dominated v1 (743us of 816us).
- The gather lands [f, pair]-transposed; a per-chunk PE is_transpose matmul
  (bf16) flips it to [pair, f], 8 chunks per PSUM bank, drained to SBUF by
  one DVE copy per bank.
- Aggregation as in v1: scaled one-hot OH~[p, (k,a)] built on DVE (bf16),
  PSUM[f, (k,a)] += E.T @ OH~ per tile; W applied per-atom post-aggregation
  (linearity), bias as count*b; norms; PE transposes; output DMA.
- All matmul inputs bf16 (PSUM accumulates fp32): rel err ~1e-3 << 2e-2.

All floating-point arithmetic runs on device. Host work is integer index
manipulation (routing/sorting/padding = sharding) and array layout.
"""

import math
from contextlib import ExitStack

import numpy as np

import concourse.bass as bass
import concourse.bacc as bacc
import concourse.tile as tile
from concourse import mybir
from concourse.bass_utils import run_bass_kernel_spmd
from concourse.masks import make_identity

F = 128
ATILE = 32  # atom window per tile
KBLK = 4  # coefficient blocks: radial, u0, u1, u2
CHUNK = 128  # pairs per matmul chunk
NBUK = 4  # sending-atom buckets (= chunks per tile)
BUK = 7500  # atoms per bucket table
SUPER_T = 6  # tiles per super-iteration
N_ATOMS = 30000


def _ap(t_ap, free_dims, off=0):
    """Custom AP view over the same partitions as t_ap with given free dims."""
    return bass.AP(t_ap.tensor, t_ap.offset + off, [t_ap.ap[0]] + list(free_dims))


def build_nc(T, n_cores):
    """Build the SPMD program for one core with T pair tiles (T % 12 == 0)."""
    CH = T * NBUK  # chunks per core
    TOTP = CH * CHUNK  # padded pair slots per core
    UW = T * 3 * ATILE  # U region width, (t, c, a) order
    OUTR = T * ATILE  # output rows (tile-slot major, host depads)
    NBLK = OUTR // 128
    VW = NBLK * 128
    SUPW = SUPER_T * NBUK * CHUNK  # pair slots per super (3072)
    GW = SUPER_T * CHUNK  # slots per gather call (768)

    fp = mybir.dt.float32
    bf = mybir.dt.bfloat16

    nc = bacc.Bacc("TRN2", target_bir_lowering=False, debug=False,
                   num_devices=n_cores)

    embT = nc.dram_tensor("embT", [F, N_ATOMS], fp, kind="ExternalInput")
    jdx = nc.dram_tensor("jdx", [128, TOTP // 16], mybir.dt.uint16,
                         kind="ExternalInput")
    fT = nc.dram_tensor("fT", [128, CH], fp, kind="ExternalInput")
    r0T = nc.dram_tensor("r0T", [128, CH], fp, kind="ExternalInput")
    r1T = nc.dram_tensor("r1T", [128, CH], fp, kind="ExternalInput")
    r2T = nc.dram_tensor("r2T", [128, CH], fp, kind="ExternalInput")
    iT = nc.dram_tensor("iT", [128, CH], fp, kind="ExternalInput")
    cnt3 = nc.dram_tensor("cnt3", [1, UW], fp, kind="ExternalInput")
    wT = nc.dram_tensor("wT", [F, F], fp, kind="ExternalInput")
    brow = nc.dram_tensor("brow", [1, F], fp, kind="ExternalInput")
    out = nc.dram_tensor("out", [OUTR, 2 * F], fp, kind="ExternalOutput")

    with tile.TileContext(nc) as tc, ExitStack() as ctx:
        cpool = ctx.enter_context(tc.tile_pool(name="const", bufs=1))
        mpool = ctx.enter_context(tc.tile_pool(name="main", bufs=1))

        # --- constants ---
        iota32 = cpool.tile([128, ATILE], fp)
        nc.gpsimd.iota(iota32[:], [[1, ATILE]], base=0, channel_multiplier=0,
                       allow_small_or_imprecise_dtypes=True)
        ident_bf = cpool.tile([128, 128], bf)
        make_identity(nc, ident_bf[:])
        ident_f = cpool.tile([128, 128], fp)
        make_identity(nc, ident_f[:])
        wT_bf = cpool.tile([F, F], bf)
        brow_bf = cpool.tile([1, F], bf)
        with tc.tile_pool(name="wld", bufs=1) as wp:
            wT_sb = wp.tile([F, F], fp)
            nc.sync.dma_start(out=wT_sb[:], in_=wT[:])
            nc.scalar.copy(wT_bf[:], wT_sb[:])
            brow_sb = wp.tile([1, F], fp)
            nc.sync.dma_start(out=brow_sb[:], in_=brow[:])
            nc.scalar.copy(brow_bf[:], brow_sb[:])

        # --- resident bf16 embedding tables (4 buckets side by side) ---
        emb_bf = mpool.tile([128, NBUK * BUK], bf)
        with tc.tile_pool(name="eld", bufs=2) as lp:
            for b in range(NBUK):
                sl = lp.tile([128, BUK], fp, tag="slab")
                nc.sync.dma_start(out=sl[:], in_=embT[:, b * BUK:(b + 1) * BUK])
                nc.scalar.copy(emb_bf[:, b * BUK:(b + 1) * BUK], sl[:])

        jdx_sb = mpool.tile([128, TOTP // 16], mybir.dt.uint16)
        nc.sync.dma_start(out=jdx_sb[:], in_=jdx[:])

        # --- persistent regions ---
        iT_sb = mpool.tile([128, CH], fp)
        nc.sync.dma_start(out=iT_sb[:], in_=iT[:])
        C_all = mpool.tile([128, CH * KBLK], bf)
        U = mpool.tile([128, UW], bf)  # uw segment sums, (t, c, a)
        R2 = mpool.tile([128, VW], fp)  # radial sums, slot-major

        # --- Phase 1: batched coefficients C[p, ch, k] (scoped scratch) ---
        with tc.tile_pool(name="p1", bufs=1) as p1:
            fT_sb = p1.tile([128, CH], fp)
            nc.sync.dma_start(out=fT_sb[:], in_=fT[:])
            r0_sb = p1.tile([128, CH], fp)
            nc.sync.dma_start(out=r0_sb[:], in_=r0T[:])
            r1_sb = p1.tile([128, CH], fp)
            nc.sync.dma_start(out=r1_sb[:], in_=r1T[:])
            r2_sb = p1.tile([128, CH], fp)
            nc.sync.dma_start(out=r2_sb[:], in_=r2T[:])
            tA = p1.tile([128, CH], fp)
            tB = p1.tile([128, CH], fp)
            mult, add = mybir.AluOpType.mult, mybir.AluOpType.add
            nc.vector.tensor_tensor(out=tA[:], in0=r0_sb[:], in1=r0_sb[:], op=mult)
            nc.vector.tensor_tensor(out=tB[:], in0=r1_sb[:], in1=r1_sb[:], op=mult)
            nc.vector.tensor_tensor(out=tA[:], in0=tA[:], in1=tB[:], op=add)
            nc.vector.tensor_tensor(out=tB[:], in0=r2_sb[:], in1=r2_sb[:], op=mult)
            nc.vector.tensor_tensor(out=tA[:], in0=tA[:], in1=tB[:], op=add)
            nc.scalar.sqrt(tA[:], tA[:])  # |r|
            nc.vector.reciprocal(tB[:], tA[:])  # 1/|r|
            nc.vector.tensor_tensor(out=tB[:], in0=fT_sb[:], in1=tB[:], op=mult)
            cview = lambda k: _ap(C_all[:], [[KBLK, CH]], off=k)
            nc.vector.tensor_copy(cview(0), fT_sb[:])
            nc.vector.tensor_tensor(out=cview(1), in0=tB[:], in1=r0_sb[:], op=mult)
            nc.vector.tensor_tensor(out=cview(2), in0=tB[:], in1=r1_sb[:], op=mult)
            nc.vector.tensor_tensor(out=cview(3), in0=tB[:], in1=r2_sb[:], op=mult)

        # --- Phases 2-5 interleaved ---
        n_super = T // SUPER_T
        MMW = 512
        NP3 = math.ceil(UW / MMW)
        TPB = 128 // ATILE
        mult, add = mybir.AluOpType.mult, mybir.AluOpType.add
        with tc.tile_pool(name="gsup", bufs=3) as gpool, \
             tc.tile_pool(name="esup", bufs=2) as epool, \
             tc.tile_pool(name="osup", bufs=2) as opool, \
             tc.tile_pool(name="ptr2", bufs=2, space="PSUM") as ptpool, \
             tc.tile_pool(name="pacc", bufs=2, space="PSUM") as ppool, \
             tc.tile_pool(name="c3", bufs=2) as c3pool, \
             tc.tile_pool(name="pw", bufs=2, space="PSUM") as wpool, \
             tc.tile_pool(name="p4", bufs=2) as p4, \
             tc.tile_pool(name="ptr", bufs=2, space="PSUM") as tpool, \
             tc.tile_pool(name="ob", bufs=2) as obpool:

            def emit_p3(k):
                c0 = k * MMW
                ncol = min(MMW, UW - c0)
                c3t = c3pool.tile([1, MMW], bf, tag="c3")
                c3f = c3pool.tile([1, MMW], fp, tag="c3f")
                nc.sync.dma_start(out=c3f[:1, :ncol],
                                  in_=cnt3[:1, c0:c0 + ncol])
                nc.scalar.copy(c3t[:1, :ncol], c3f[:1, :ncol])
                pw = wpool.tile([128, MMW], fp, tag="pw")
                nc.tensor.matmul(out=pw[:, :ncol], lhsT=wT_bf[:],
                                 rhs=U[:, c0:c0 + ncol], start=True,
                                 stop=False)
                nc.tensor.matmul(out=pw[:, :ncol], lhsT=brow_bf[:1, :],
                                 rhs=c3t[:1, :ncol], start=False, stop=True)
                nc.scalar.copy(U[:, c0:c0 + ncol], pw[:, :ncol])

            def emit_blk(blk):
                t0 = blk * TPB
                s0 = p4.tile([128, 128], fp, tag="s0")
                s1 = p4.tile([128, 128], fp, tag="s1")
                vb = p4.tile([128, 128], fp, tag="vb")
                uvw = lambda c: _ap(U[:], [[3 * ATILE, TPB], [1, ATILE]],
                                    off=t0 * 3 * ATILE + c * ATILE)
                sv0 = _ap(s0[:], [[ATILE, TPB], [1, ATILE]])
                sv1 = _ap(s1[:], [[ATILE, TPB], [1, ATILE]])
                nc.vector.tensor_tensor(out=sv0, in0=uvw(0), in1=uvw(0), op=mult)
                nc.vector.tensor_tensor(out=sv1, in0=uvw(1), in1=uvw(1), op=mult)
                nc.vector.tensor_tensor(out=sv0, in0=s0[:], in1=s1[:], op=add)
                nc.vector.tensor_tensor(out=sv1, in0=uvw(2), in1=uvw(2), op=mult)
                nc.vector.tensor_tensor(out=sv0, in0=s0[:], in1=s1[:], op=add)
                nc.scalar.sqrt(vb[:], s0[:])
                ob = obpool.tile([128, 2 * F], fp, tag="ob")
                pt = tpool.tile([128, 128], fp, tag="pt")
                nc.tensor.matmul(out=pt[:], lhsT=vb[:],
                                 rhs=ident_f[:], is_transpose=True,
                                 start=True, stop=True)
                nc.scalar.copy(ob[:, 0:F], pt[:])
                pt2 = tpool.tile([128, 128], fp, tag="pt")
                nc.tensor.matmul(out=pt2[:], lhsT=R2[:, blk * 128:(blk + 1) * 128],
                                 rhs=ident_f[:], is_transpose=True,
                                 start=True, stop=True)
                nc.scalar.copy(ob[:, F:2 * F], pt2[:])
                nc.sync.dma_start(out=out[blk * 128:(blk + 1) * 128, :],
                                  in_=ob[:])

            g_tiles = {}

            def do_gather(s):
                sup0 = s * SUPW
                g = gpool.tile([128, SUPW], bf, tag="gsup")
                for b in range(NBUK):
                    nc.gpsimd.indirect_copy(
                        g[:, b * GW:(b + 1) * GW],
                        emb_bf[:, b * BUK:(b + 1) * BUK],
                        jdx_sb[:, (sup0 + b * GW) // 16:
                               (sup0 + (b + 1) * GW) // 16],
                        True,
                    )
                g_tiles[s] = g

            next_p3 = 0
            next_blk = 0
            do_gather(0)
            for s in range(n_super):
                if s + 1 < n_super:
                    do_gather(s + 1)
                g_sup = g_tiles.pop(s)

                ch0 = s * SUPER_T * NBUK
                sc = SUPER_T * NBUK
                oh_sup = opool.tile([128, sc * ATILE], bf, tag="ohsup")
                ot_sup = opool.tile([128, sc * F], bf, tag="otsup")
                nc.vector.tensor_tensor(
                    out=_ap(oh_sup[:], [[ATILE, sc], [1, ATILE]]),
                    in0=_ap(iT_sb[:], [[1, sc], [0, ATILE]], off=ch0),
                    in1=_ap(iota32[:], [[0, sc], [1, ATILE]]),
                    op=mybir.AluOpType.is_equal,
                )
                nc.vector.tensor_tensor(
                    out=_ap(ot_sup[:], [[F, sc], [ATILE, KBLK], [1, ATILE]]),
                    in0=_ap(oh_sup[:], [[ATILE, sc], [0, KBLK], [1, ATILE]]),
                    in1=_ap(C_all[:], [[KBLK, sc], [1, KBLK], [0, ATILE]],
                            off=ch0 * KBLK),
                    op=mybir.AluOpType.mult,
                )

                # transpose gathered [f, p] chunks to [p, f]; groups of 6
                # chunks aligned to the per-bucket indirect_copy slices so
                # each group only depends on its own gather
                e_sup = epool.tile([128, sc * F], bf, tag="esup")
                for q in range(NBUK):
                    pt6 = ptpool.tile([128, SUPER_T * 128], bf, tag="pt6")
                    for c in range(SUPER_T):
                        ch = q * SUPER_T + c
                        nc.tensor.matmul(
                            out=pt6[:, c * 128:(c + 1) * 128],
                            lhsT=g_sup[:, ch * 128:(ch + 1) * 128],
                            rhs=ident_bf[:], is_transpose=True,
                            start=True, stop=True,
                        )
                    nc.vector.tensor_copy(
                        e_sup[:, q * SUPER_T * 128:(q + 1) * SUPER_T * 128],
                        pt6[:])

                for ti in range(SUPER_T):
                    t = s * SUPER_T + ti
                    acc = ppool.tile([128, F], fp, tag="acc")
                    for b in range(NBUK):
                        g = b * SUPER_T + ti
                        nc.tensor.matmul(
                            out=acc[:],
                            lhsT=_ap(e_sup[:], [[1, F]], off=g * F),
                            rhs=_ap(ot_sup[:], [[1, F]], off=g * F),
                            start=(b == 0),
                            stop=(b == NBUK - 1),
                        )
                    nc.scalar.copy(R2[:, t * ATILE:(t + 1) * ATILE],
                                   acc[:, 0:ATILE])
                    nc.scalar.copy(U[:, t * 3 * ATILE:(t + 1) * 3 * ATILE],
                                   acc[:, ATILE:F])

                # trailing work whose inputs are now flushed
                flushed_cols = (s + 1) * SUPER_T * 3 * ATILE
                while next_p3 < NP3 and (
                        (next_p3 + 1) * MMW <= flushed_cols
                        or s + 1 == n_super):
                    emit_p3(next_p3)
                    next_p3 += 1
                while next_blk < NBLK and (next_blk + 1) * TPB * 3 * ATILE \
                        <= next_p3 * MMW:
                    emit_blk(next_blk)
                    next_blk += 1
            while next_blk < NBLK:
                emit_blk(next_blk)
                next_blk += 1

    nc.compile()
    return nc


def host_prep(inputs, n_cores=8):
    """Route pairs to atom-owning cores; 32-atom tiles with 4 bucket-chunks."""
    emb = np.ascontiguousarray(np.asarray(inputs["atomic_embedding"],
                                          dtype=np.float32))
    f = np.asarray(inputs["f_ij_cutoff"], dtype=np.float32).ravel()
    r = np.asarray(inputs["r_ij"], dtype=np.float32)
    W = np.asarray(inputs["W"], dtype=np.float32)
    b = np.asarray(inputs["b"], dtype=np.float32)
    pl = np.asarray(inputs["pairlist"]).astype(np.int64)
    idx_i, idx_j = pl[0], pl[1]

    N = emb.shape[0]
    P = idx_i.shape[0]
    APC = N // n_cores
    SLOTS = NBUK * CHUNK  # pair slots per tile (512)
    buk_of_pair = idx_j // BUK

    # per-atom-per-bucket counts [N, NBUK]
    cnt_ab = np.zeros((N, NBUK), dtype=np.int64)
    np.add.at(cnt_ab, (idx_i, buk_of_pair), 1)
    cnt_atom = cnt_ab.sum(axis=1)

    # greedy variable-base tiling per core: close a tile when any bucket
    # chunk (128 slots) would overflow or the atom window exceeds ATILE
    tiles = []
    for c in range(n_cores):
        ca = cnt_ab[c * APC:(c + 1) * APC]
        tl = []
        cur = 0
        cur_b = np.zeros(NBUK, dtype=np.int64)
        for a in range(APC):
            cb = ca[a]
            if (cur_b + cb > CHUNK).any() or a - cur >= ATILE:
                tl.append((cur, a))
                cur = a
                cur_b[:] = 0
            cur_b += cb
        tl.append((cur, APC))
        tiles.append(tl)
    T = max(len(tl) for tl in tiles)
    T = ((T + 11) // 12) * 12  # multiple of SUPER_T(6) and 4

    tile_of_atom = np.zeros(N, dtype=np.int64)
    base_of_atom = np.zeros(N, dtype=np.int64)
    for c in range(n_cores):
        for t, (a0, a1) in enumerate(tiles[c]):
            tile_of_atom[c * APC + a0:c * APC + a1] = t
            base_of_atom[c * APC + a0:c * APC + a1] = a0

    # slot layout: super s => [bucket b => [tile ti => 128 slots]]
    # global chunk index g = s*24 + b*6 + ti ; slot = g*128 + pos
    order = np.argsort(idx_i, kind="stable")
    so_i = idx_i[order]
    so_b = buk_of_pair[order]
    core_of = so_i // APC
    t_of = tile_of_atom[so_i]
    s_of = t_of // SUPER_T
    ti_of = t_of % SUPER_T
    chunk_of = s_of * (SUPER_T * NBUK) + so_b * SUPER_T + ti_of
    key = core_of * (T * NBUK) + chunk_of
    cnt = np.bincount(key, minlength=n_cores * T * NBUK)
    assert cnt.max() <= CHUNK, cnt.max()
    starts = np.zeros(n_cores * T * NBUK + 1, dtype=np.int64)
    np.cumsum(cnt, out=starts[1:])
    # order within (core, chunk): stable sort by key keeps i-sorted order
    order2 = np.argsort(key, kind="stable")
    pos = np.arange(P, dtype=np.int64)
    pos_in_chunk = pos - starts[key[order2]]
    slot = np.empty(P, dtype=np.int64)
    slot[order2] = key[order2] * CHUNK + pos_in_chunk
    TOT = n_cores * T * NBUK * CHUNK

    jj = np.zeros(TOT, dtype=np.uint16)
    ff = np.zeros(TOT, dtype=np.float32)
    rr = np.zeros((TOT, 3), dtype=np.float32)
    rr[:, 0] = 1.0
    ii = np.zeros(TOT, dtype=np.float32)
    jj[slot] = (idx_j[order] - so_b * BUK).astype(np.uint16)
    ff[slot] = f[order]
    rr[slot] = r[order]
    ii[slot] = (so_i - core_of * APC - base_of_atom[so_i]).astype(np.float32)

    TOTC = T * NBUK * CHUNK  # padded pair slots per core
    CH = T * NBUK
    in_maps = []
    out_sel = []
    embT = np.ascontiguousarray(emb.T)
    wTc = np.ascontiguousarray(W.T)
    browc = np.ascontiguousarray(b.reshape(1, F))
    for c in range(n_cores):
        sl = slice(c * TOTC, (c + 1) * TOTC)
        jj_c = jj[sl]
        a16 = np.ascontiguousarray(jj_c.reshape(TOTC // 16, 16).T)
        jdx = np.ascontiguousarray(np.tile(a16, (8, 1)))
        tr = lambda x: np.ascontiguousarray(x.reshape(CH, CHUNK).T)
        cnt3 = np.zeros((T, 3, ATILE), dtype=np.float32)
        rows_slot = []
        rows_atom = []
        for t, (a0, a1) in enumerate(tiles[c]):
            span = a1 - a0
            cnt3[t, :, :span] = cnt_atom[c * APC + a0:c * APC + a1][None, :]
            rows_slot.append(np.arange(t * ATILE, t * ATILE + span))
            rows_atom.append(np.arange(c * APC + a0, c * APC + a1))
        out_sel.append((np.concatenate(rows_slot), np.concatenate(rows_atom)))
        in_maps.append({
            "embT": embT,
            "jdx": jdx,
            "fT": tr(ff[sl]),
            "r0T": tr(rr[sl][:, 0]),
            "r1T": tr(rr[sl][:, 1]),
            "r2T": tr(rr[sl][:, 2]),
            "iT": tr(ii[sl]),
            "cnt3": np.ascontiguousarray(cnt3.reshape(1, -1)),
            "wT": wTc,
            "brow": browc,
        })
    return in_maps, dict(N=N, APC=APC, T=T, P=P, out_sel=out_sel)


_NC_CACHE = {}


def kernel(**inputs) -> np.ndarray:
    n_cores = 8
    in_maps, meta = host_prep(inputs, n_cores)
    N = meta["N"]
    ckey = (N, meta["T"], n_cores)
    nc = _NC_CACHE.get(ckey)
    if nc is None:
        nc = build_nc(meta["T"], n_cores)
        _NC_CACHE[ckey] = nc
    res = run_bass_kernel_spmd(nc, in_maps, core_ids=list(range(n_cores)))
    out = np.empty((N, 2 * F), dtype=np.float32)
    for c in range(n_cores):
        rows_slot, rows_atom = meta["out_sel"][c]
        out[rows_atom] = res.results[c]["out"][rows_slot]
    return out
